# revision 1
# baseline (speedup 1.0000x reference)
"""Trainium2 Bass kernel for the GAT block (masked attention + SwiGLU MLP).

Sharding: token-split across 8 cores. Core c handles batch b = c//4 and the
512-query slice starting at (c%4)*512 of that batch. Each core computes
full-batch K/V projections (duplicated across the 4 cores of a batch -- no
collectives), its own queries' attention, and the MLP for its token slice.

Device-side strategy:
  - activations token-major [tokens, d] for normalizations (free-dim
    reductions, per-partition scales), PE-transposed to feature-major
    [d, tokens] where they feed matmul contractions.
  - attention scores computed TRANSPOSED: sT[keys, queries], so exp/mask
    need no reductions and the AV matmul consumes p = exp(sT)*mask directly
    as the moving operand (no [N,N] transposes).
  - softmax denominators ride along as a 65th output row via a ones column
    appended to V; normalization + residual fused after AV on [65,*] tiles.
  - no max-subtraction in softmax: scores are O(6) for this input
    distribution, exp is safe, softmax is shift-invariant.
  - host folds g1/g2 into weights, 1/sqrt(hd) into Wq/bq, bv into the
    attention residual; remaining biases fold into per-partition
    PSUM-evacuation activations.
"""

import os
import sys

sys.path.insert(0, "/opt/trn_rl_repo")

# CoreSim doesn't implement Silu; sim runs decompose it into Sigmoid+mul.
SIM_SILU = os.environ.get("KSIM_SILU") == "1"
# weight/activation compute dtype for projections+MLP: f32r (default) or bf16
KDT = os.environ.get("KDT", "f32r")

from contextlib import ExitStack

import ml_dtypes
import numpy as np

import concourse.bass as bass
import concourse.mybir as mybir
import concourse.tile as tile
from concourse import bacc
from concourse.masks import make_identity

D = 512
N = 2048
B = 2
HEADS = 8
HD = 64
HDIM = 2048
NCORES = 8
QT = 512  # tokens (queries) per core
EPS = float(np.finfo(np.float32).eps)

F32 = mybir.dt.float32
F32R = mybir.dt.float32r
BF16 = mybir.dt.bfloat16

PROJ_DT = F32R  # q/k/v projection matmuls
MLP_DT = F32R   # W1/W2/W3 matmuls

WDT = None  # set below
AF = mybir.ActivationFunctionType
ALU = mybir.AluOpType

WDT = BF16 if KDT == "bf16" else F32R
ZTDT = BF16 if KDT == "bf16" else F32  # pre-transpose z tiles / transpose PSUM

DT4 = D // 128    # 4 feature tiles
TT = N // 128     # 16 token tiles (full batch)
QTT = QT // 128   # 4 own-query tiles
HT = HDIM // 128  # 16 hidden tiles
KC = N // 512     # 4 key chunks of 512


def _mm(nc, out, lhsT, rhs, dt, **kw):
    nc.tensor.matmul(out, lhsT, rhs, **kw)


def build_module(reps=1, stage=4):
    # stage: 1=input DMAs only, 2=+front+projections, 3=+attention, 4=full
    nc = bacc.Bacc(
        "TRN2", target_bir_lowering=False, debug=False, num_devices=NCORES)

    p = {}
    def param(name, shape, dtype=F32, out=False):
        p[name] = nc.declare_dram_parameter(name, shape, dtype, isOutput=out)
        return p[name]

    param("xf", [N, D])            # full batch x
    param("xo", [QT, D])           # own-slice x
    param("xb", [QT, D])           # own-slice x + bv (residual base)
    param("mT", [N, QT], BF16)     # mask transposed [keys, queries], 0/1
    param("wqT", [D, D], WDT)           # (Wq*g1).T / 8
    param("bq8", [D, 1])           # bq / 8
    param("wkT", [D, D], WDT)           # (Wk*g1).T
    param("bk", [D, 1])
    param("wvT", [D, D], WDT)           # (Wv*g1).T
    param("w1T", [D, HDIM], WDT)        # (W1*g2).T
    param("b1", [HDIM, 1])
    param("w2T", [D, HDIM], WDT)        # (W2*g2).T
    param("b2", [HDIM, 1])
    param("w3T", [HDIM, D], WDT)        # W3.T
    param("b3", [D, 1])
    param("out", [QT, D], out=True)

    with ExitStack() as ctx:
        tc = ctx.enter_context(tile.TileContext(nc))
        for _ in range(reps):
            with ExitStack() as rctx:
                _body(rctx, tc, nc, p, stage)
    nc.compile()
    return nc


def _body(ctx, tc, nc, p, stage=4):
    # ---------- long-lived pools ----------
    persist = ctx.enter_context(tc.tile_pool(name="persist", bufs=1))
    small = ctx.enter_context(tc.tile_pool(name="small", bufs=8))

    ident = persist.tile([128, 128], F32, tag="ident", name="ident")
    make_identity(nc, ident[:])
    identw = ident
    if WDT == BF16:
        identw = persist.tile([128, 128], BF16, tag="identw", name="identw")
        nc.vector.tensor_copy(identw[:], ident[:])
    epsb = persist.tile([128, 1], F32, tag="epsb", name="epsb")
    nc.gpsimd.memset(epsb[:], EPS)

    xb_s = [persist.tile([128, D], F32, tag=f"xb{q}", name=f"xb{q}") for q in range(QTT)]
    for q in range(QTT):
        nc.sync.dma_start(xb_s[q][:], p["xb"][q * 128:(q + 1) * 128, :])
    hbuf = [persist.tile([128, D], F32, tag=f"hb{q}", name=f"hb{q}") for q in range(QTT)]

    def rms_tile(front, x_ap):
        """inv_rms [128,1] for a token-major [128, D] tile."""
        scr = front.tile([128, D], BF16, tag="rms_scr", name="rms_scr")
        ssq = small.tile([128, 1], F32, tag="ssq", name="ssq")
        nc.vector.scalar_tensor_tensor(
            out=scr[:], in0=x_ap, scalar=1.0, in1=x_ap,
            op0=ALU.mult, op1=ALU.mult, accum_out=ssq[:])
        srt = small.tile([128, 1], F32, tag="srt", name="srt")
        nc.scalar.activation(srt[:], ssq[:], AF.Sqrt, bias=epsb[:], scale=1.0 / D)
        inv = small.tile([128, 1], F32, tag="inv", name="inv")
        nc.vector.reciprocal(inv[:], srt[:])
        return inv

    # ================= scope 1: front (z, zT, projections' sources) ========
    s1 = ExitStack()
    wpool = s1.enter_context(tc.tile_pool(name="wqkv", bufs=1))
    front = s1.enter_context(tc.tile_pool(name="front", bufs=4))
    mm_ps = s1.enter_context(tc.tile_pool(name="mm_ps", bufs=3, space="PSUM"))

    wq_s = [wpool.tile([128, D], WDT, tag=f"wq{i}", name=f"wq{i}") for i in range(DT4)]
    wk_s = [wpool.tile([128, D], WDT, tag=f"wk{i}", name=f"wk{i}") for i in range(DT4)]
    wv_s = [wpool.tile([128, D], WDT, tag=f"wv{i}", name=f"wv{i}") for i in range(DT4)]
    for i in range(DT4):
        nc.sync.dma_start(wq_s[i][:], p["wqT"][i * 128:(i + 1) * 128, :])
        nc.sync.dma_start(wk_s[i][:], p["wkT"][i * 128:(i + 1) * 128, :])
        nc.sync.dma_start(wv_s[i][:], p["wvT"][i * 128:(i + 1) * 128, :])
    bq_s = [small.tile([128, 1], F32, tag=f"bqs{i}", name=f"bqs{i}") for i in range(DT4)]
    bk_s = [small.tile([128, 1], F32, tag=f"bks{i}", name=f"bks{i}") for i in range(DT4)]
    for i in range(DT4):
        nc.sync.dma_start(bq_s[i][:], p["bq8"][i * 128:(i + 1) * 128, :])
        nc.sync.dma_start(bk_s[i][:], p["bk"][i * 128:(i + 1) * 128, :])

    # single tensors, d-major chunks: zT_all[:, d*N + col], zoT_all[:, d*QT + col]
    zT_all = wpool.tile([128, DT4 * N], WDT, tag="zT_all", name="zT_all")
    zoT_all = wpool.tile([128, DT4 * QT], WDT, tag="zoT_all", name="zoT_all")
    zT = [zT_all[:, d * N:(d + 1) * N] for d in range(DT4)]
    zoT = [zoT_all[:, d * QT:(d + 1) * QT] for d in range(DT4)]

    def norm_transpose(x_dram, row0, ntiles, zT_dst_all, ncols, col0):
        """token-major rows -> normalized + transposed; rms scalar chains
        batched per 4-tile group (one sqrt + one reciprocal), one batched
        PSUM bank of 4 transposes + single strided evacuation per tile."""
        G = 4
        assert ntiles % G == 0
        for t in []:
            pass
        for g in range(ntiles // G):
            sss = small.tile([128, G], F32, tag="sss", name="sss")
            srtg = small.tile([128, G], F32, tag="srtg", name="srtg")
            invg = small.tile([128, G], F32, tag="invg", name="invg")
            xts = []
            for i in range(G):
                t = g * G + i
                xt = front.tile([128, D], F32, tag="xt", name="xt")
                nc.sync.dma_start(xt[:], x_dram[row0 + t * 128:row0 + (t + 1) * 128, :])
                scr = front.tile([128, D], BF16, tag="rms_scr", name="rms_scr")
                nc.vector.scalar_tensor_tensor(
                    out=scr[:], in0=xt[:], scalar=1.0, in1=xt[:],
                    op0=ALU.mult, op1=ALU.mult, accum_out=sss[:, i:i + 1])
                xts.append(xt)
            nc.scalar.activation(srtg[:], sss[:], AF.Sqrt, bias=epsb[:], scale=1.0 / D)
            nc.vector.reciprocal(invg[:], srtg[:])
            for i in range(G):
                t = g * G + i
                _norm_tile(xts[i], invg[:, i:i + 1], t, zT_dst_all, ncols, col0)

    def _norm_tile(xt, inv_ap, t, zT_dst_all, ncols, col0):
        if True:
            zt = front.tile([128, D], ZTDT, tag="zt", name="zt")
            nc.gpsimd.tensor_scalar_mul(zt[:], xt[:], inv_ap)
            ps = mm_ps.tile([128, 512], ZTDT, tag="mm", name="mm")
            for d in range(DT4):
                nc.tensor.matmul(ps[:, d * 128:(d + 1) * 128],
                                 zt[:, d * 128:(d + 1) * 128], identw[:],
                                 is_transpose=True,
                                 start=(d == 0), stop=(d == DT4 - 1))
            dst = zT_dst_all[:, col0:].rearrange(
                "p (d c) -> p d c", d=DT4, c=ncols)[:, :, 0:128] \
                if False else zT_dst_all[:].rearrange(
                "p (d c) -> p d c", c=ncols)[:, :, col0 + t * 128:col0 + (t + 1) * 128]
            eng = nc.scalar.copy if t % 2 == 0 else nc.vector.tensor_copy
            eng(dst, ps[:].rearrange("p (d c) -> p d c", c=128))

    if stage == 1:
        # DMA-only measurement: emit all input loads, no compute.
        for t in range(TT):
            xt = front.tile([128, D], F32, tag="xt", name="xt")
            nc.sync.dma_start(xt[:], p["xf"][t * 128:(t + 1) * 128, :])
        for t in range(QTT):
            xt = front.tile([128, D], F32, tag="xt", name="xt")
            nc.sync.dma_start(xt[:], p["xo"][t * 128:(t + 1) * 128, :])
        mtd = [wpool.tile([128, QT], BF16, tag=f"mtd{k}", name=f"mtd{k}") for k in range(TT)]
        for k in range(TT):
            nc.sync.dma_start(mtd[k][:], p["mT"][k * 128:(k + 1) * 128, :])
        s1.close()
        sdma = ExitStack()
        wdp = sdma.enter_context(tc.tile_pool(name="wdp", bufs=1))
        w1d = [wdp.tile([128, HDIM], WDT, tag=f"w1d{i}", name=f"w1d{i}") for i in range(DT4)]
        w2d = [wdp.tile([128, HDIM], WDT, tag=f"w2d{i}", name=f"w2d{i}") for i in range(DT4)]
        for i in range(DT4):
            nc.sync.dma_start(w1d[i][:], p["w1T"][i * 128:(i + 1) * 128, :])
            nc.sync.dma_start(w2d[i][:], p["w2T"][i * 128:(i + 1) * 128, :])
        w3d = [wdp.tile([128, D], WDT, tag=f"w3d{j}", name=f"w3d{j}") for j in range(HT)]
        for j in range(HT):
            nc.sync.dma_start(w3d[j][:], p["w3T"][j * 128:(j + 1) * 128, :])
        for qc in range(QTT):
            nc.sync.dma_start(p["out"][qc * 128:(qc + 1) * 128, :], xb_s[qc][:])
        sdma.close()
        return

    norm_transpose(p["xf"], 0, TT, zT_all, N, 0)
    norm_transpose(p["xo"], 0, QTT, zoT_all, QT, 0)

    # ---------- scope 2 pools (attention operands, produced here) ----------
    s2 = ExitStack()
    apool = s2.enter_context(tc.tile_pool(name="attn", bufs=1, side="right"))
    arot = s2.enter_context(tc.tile_pool(name="arot", bufs=4, side="right"))

    kT = [apool.tile([128, N], BF16, tag=f"kT{pr}", name=f"kT{pr}") for pr in range(DT4)]
    qT = [apool.tile([128, QT], BF16, tag=f"qT{pr}", name=f"qT{pr}") for pr in range(DT4)]
    v65_all = apool.tile([128, TT * HEADS * (HD + 1)], BF16, tag="v65_all", name="v65_all")
    v65 = [v65_all[:, t * HEADS * (HD + 1):(t + 1) * HEADS * (HD + 1)] for t in range(TT)]
    mt2 = [apool.tile([128, 2 * QT], BF16, tag=f"mt2_{g}", name=f"mt2_{g}")
           for g in range(TT // 2)]
    for g in range(TT // 2):
        nc.sync.dma_start(
            mt2[g][:].rearrange("p (a q) -> p a q", a=2),
            p["mT"][g * 256:(g + 1) * 256, :].rearrange("(a p) q -> p a q", p=128))

    # kT projection (full batch): 4 chunk-groups in one 4-bank PSUM, one evac
    for pr in range(DT4):
        ps = mm_ps.tile([128, 2048], F32, tag="pk", name="pk", bufs=1)
        for c4 in range(KC):
            for dk in range(DT4):
                _mm(nc, ps[:, c4 * 512:(c4 + 1) * 512],
                    wk_s[dk][:, pr * 128:(pr + 1) * 128],
                    zT[dk][:, c4 * 512:(c4 + 1) * 512], PROJ_DT,
                    start=(dk == 0), stop=(dk == DT4 - 1))
        nc.scalar.activation(kT[pr][:], ps[:], AF.Identity,
                             bias=bk_s[pr][:], scale=1.0)
    # qT projection (own slice)
    for pr in range(DT4):
        ps = mm_ps.tile([128, 512], F32, tag="mm", name="mm")
        for dk in range(DT4):
            _mm(nc, ps[:], wq_s[dk][:, pr * 128:(pr + 1) * 128], zoT[dk][:], PROJ_DT,
                start=(dk == 0), stop=(dk == DT4 - 1))
        nc.scalar.activation(qT[pr][:], ps[:], AF.Identity, bias=bq_s[pr][:], scale=1.0)
    # v projection (token-major, full batch) -> v65; grouped 4 token tiles
    nc.vector.memset(
        v65_all[:].rearrange("q (t h c) -> q t h c", t=TT, c=HD + 1)[:, :, :, HD:HD + 1],
        1.0)
    for g4 in range(TT // 4):
        ps = mm_ps.tile([128, 2048], F32, tag="pk", name="pk", bufs=1)
        for tt in range(4):
            t = g4 * 4 + tt
            for dk in range(DT4):
                _mm(nc, ps[:, tt * 512:(tt + 1) * 512],
                    zT[dk][:, t * 128:(t + 1) * 128], wv_s[dk][:], PROJ_DT,
                    start=(dk == 0), stop=(dk == DT4 - 1))
        dst = v65_all[:, g4 * 4 * HEADS * (HD + 1):(g4 + 1) * 4 * HEADS * (HD + 1)]
        nc.vector.tensor_copy(
            dst.rearrange("q (t h c) -> q t h c", t=4, c=HD + 1)[:, :, :, 0:HD],
            ps[:].rearrange("q (t h c) -> q t h c", t=4, c=HD))

    if stage == 2:
        for qc in range(QTT):
            nc.sync.dma_start(p["out"][qc * 128:(qc + 1) * 128, :], xb_s[qc][:])
        s1.close()
        s2.close()
        return

    s1.close()  # frees wqkv/front zones (zT, zoT, wq/wk/wv) + mm_ps banks

    s2b = ExitStack()
    sc_ps = s2b.enter_context(tc.tile_pool(name="sc_ps", bufs=3, space="PSUM", side="right"))
    av_ps = s2b.enter_context(tc.tile_pool(name="av_ps", bufs=1, space="PSUM", side="right"))
    tr_ps = s2b.enter_context(tc.tile_pool(name="tr_ps", bufs=1, space="PSUM", side="right"))

    # ---- MLP weights: load during attention into the freed zone ----
    s3 = ExitStack()
    w12pool = s3.enter_context(tc.tile_pool(name="w12", bufs=1))
    w1_s = [w12pool.tile([128, HDIM], WDT, tag=f"w1{i}", name=f"w1{i}") for i in range(DT4)]
    w2_s = [w12pool.tile([128, HDIM], WDT, tag=f"w2{i}", name=f"w2{i}") for i in range(DT4)]
    for i in range(DT4):
        nc.sync.dma_start(w1_s[i][:], p["w1T"][i * 128:(i + 1) * 128, :])
        nc.sync.dma_start(w2_s[i][:], p["w2T"][i * 128:(i + 1) * 128, :])

    # ================= attention =================
    for pr in range(DT4):  # head pairs
        p_t = [apool.tile([128, TT * 512], BF16, tag=f"p{sub}", name=f"p{sub}") for sub in (0, 1)]
        for g in range(TT // 2):  # kt groups of 2
            ps_pair = []
            for sub in (0, 1):
                ps_s = sc_ps.tile([128, 1024], F32, tag="sc", name="sc")
                ps_pair.append(ps_s)
            for half in (0, 1):
                kt = 2 * g + half
                for sub in (0, 1):
                    lhsT = kT[pr][64 * sub:64 * (sub + 1), kt * 128:(kt + 1) * 128]
                    rhs = qT[pr][64 * sub:64 * (sub + 1), :]
                    nc.tensor.matmul(ps_pair[sub][:, half * 512:(half + 1) * 512],
                                     lhsT, rhs, start=True, stop=True,
                                     tile_position=(64 * sub, 0))
            for sub in (0, 1):
                praw = arot.tile([128, 1024], BF16, tag="praw", name="praw")
                nc.scalar.activation(praw[:], ps_pair[sub][:], AF.Exp,
                                     bias=0.0, scale=1.0)
                nc.vector.tensor_mul(p_t[sub][:, g * 1024:(g + 1) * 1024],
                                     praw[:], mt2[g][:])
        for sub in (0, 1):
            h = 2 * pr + sub
            ps_o = av_ps.tile([65, 512], F32, tag="av", name="av")
            for kt in range(TT):
                nc.tensor.matmul(ps_o[:], v65[kt][:, 65 * h:65 * (h + 1)],
                                 p_t[sub][:, kt * 512:(kt + 1) * 512],
                                 start=(kt == 0), stop=(kt == TT - 1))
            oT = arot.tile([65, 512], F32, tag="oT", name="oT")
            nc.vector.tensor_copy(oT[:], ps_o[:])
            for qc in range(QTT):
                ps_t = tr_ps.tile([128, 65], F32, tag="otr", name="otr")
                nc.tensor.transpose(ps_t[:], oT[:, qc * 128:(qc + 1) * 128],
                                    ident[0:65, 0:65])
                rec = small.tile([128, 1], F32, tag="rec", name="rec")
                nc.vector.reciprocal(rec[:], ps_t[:, 64:65])
                nc.vector.scalar_tensor_tensor(
                    out=hbuf[qc][:, HD * h:HD * (h + 1)], in0=ps_t[:, 0:HD],
                    scalar=rec[:], in1=xb_s[qc][:, HD * h:HD * (h + 1)],
                    op0=ALU.mult, op1=ALU.add)

    s2b.close()
    s2.close()  # frees kT/qT/v65/mask/p zones

    if stage == 3:
        for qc in range(QTT):
            nc.sync.dma_start(p["out"][qc * 128:(qc + 1) * 128, :], hbuf[qc][:])
        s3.close()
        return

    # ================= hn + MLP =================
    s4 = ExitStack()
    mpool = s4.enter_context(tc.tile_pool(name="mlp", bufs=1))
    mrot = s4.enter_context(tc.tile_pool(name="mrot", bufs=3))
    mm_ps = s4.enter_context(tc.tile_pool(name="mm_ps2", bufs=3, space="PSUM"))
    w3_s = [mpool.tile([128, D], WDT, tag=f"w3{j}", name=f"w3{j}") for j in range(HT)]
    for j in range(HT):
        nc.sync.dma_start(w3_s[j][:], p["w3T"][j * 128:(j + 1) * 128, :])
    hnT_all = mpool.tile([128, DT4 * QT], WDT, tag="hnT_all", name="hnT_all")
    hnT = [hnT_all[:, d * QT:(d + 1) * QT] for d in range(DT4)]
    gbuf = [mpool.tile([128, QT], WDT, tag=f"g{j}", name=f"g{j}") for j in range(HT)]
    outbuf = [mpool.tile([128, D], F32, tag=f"ob{q}", name=f"ob{q}") for q in range(QTT)]

    for qc in range(QTT):
        inv2 = rms_tile(mrot, hbuf[qc][:])
        z2 = mrot.tile([128, D], ZTDT, tag="z2", name="z2")
        nc.gpsimd.tensor_scalar_mul(z2[:], hbuf[qc][:], inv2[:])
        ps = mm_ps.tile([128, 512], ZTDT, tag="mm", name="mm")
        for d in range(DT4):
            nc.tensor.matmul(ps[:, d * 128:(d + 1) * 128],
                             z2[:, d * 128:(d + 1) * 128], identw[:],
                             is_transpose=True, start=(d == 0), stop=(d == DT4 - 1))
        eng = nc.scalar.copy if qc % 2 == 0 else nc.vector.tensor_copy
        eng(hnT_all[:].rearrange("p (d c) -> p d c", c=QT)[:, :, qc * 128:(qc + 1) * 128],
            ps[:].rearrange("p (d c) -> p d c", c=128))

    for j in range(HT):
        b1t = small.tile([128, 1], F32, tag="b1t", name="b1t")
        nc.sync.dma_start(b1t[:], p["b1"][j * 128:(j + 1) * 128, :])
        b2t = small.tile([128, 1], F32, tag="b2t", name="b2t")
        nc.sync.dma_start(b2t[:], p["b2"][j * 128:(j + 1) * 128, :])
        ps2 = mm_ps.tile([128, 512], F32, tag="mm", name="mm")
        for dk in range(DT4):
            _mm(nc, ps2[:], w1_s[dk][:, j * 128:(j + 1) * 128], hnT[dk][:], MLP_DT,
                start=(dk == 0), stop=(dk == DT4 - 1))
        su = mrot.tile([128, 512], F32, tag="su", name="su")
        if SIM_SILU:
            a2 = mrot.tile([128, 512], F32, tag="a2", name="a2")
            nc.scalar.activation(a2[:], ps2[:], AF.Identity, bias=b1t[:], scale=1.0)
            sg = mrot.tile([128, 512], F32, tag="sg", name="sg")
            nc.scalar.activation(sg[:], ps2[:], AF.Sigmoid, bias=b1t[:], scale=1.0)
            nc.vector.tensor_mul(su[:], a2[:], sg[:])
        else:
            nc.scalar.activation(su[:], ps2[:], AF.Silu, bias=b1t[:], scale=1.0)
        ps3 = mm_ps.tile([128, 512], F32, tag="mm", name="mm")
        for dk in range(DT4):
            _mm(nc, ps3[:], w2_s[dk][:, j * 128:(j + 1) * 128], hnT[dk][:], MLP_DT,
                start=(dk == 0), stop=(dk == DT4 - 1))
        nc.vector.scalar_tensor_tensor(
            out=gbuf[j][:], in0=ps3[:], scalar=b2t[:], in1=su[:],
            op0=ALU.add, op1=ALU.mult)

    for i in range(DT4):
        b3t = small.tile([128, 1], F32, tag="b3t", name="b3t")
        nc.sync.dma_start(b3t[:], p["b3"][i * 128:(i + 1) * 128, :])
        ps4 = mm_ps.tile([128, 512], F32, tag="mm", name="mm")
        for j in range(HT):
            _mm(nc, ps4[:], w3_s[j][:, i * 128:(i + 1) * 128], gbuf[j][:], MLP_DT,
                start=(j == 0), stop=(j == HT - 1))
        outT = mrot.tile([128, 512], F32, tag="outT", name="outT")
        nc.scalar.activation(outT[:], ps4[:], AF.Identity, bias=b3t[:], scale=1.0)
        for qc in range(QTT):
            ps5 = mm_ps.tile([128, 128], F32, tag="mm", name="mm")
            nc.tensor.transpose(ps5[:], outT[:, qc * 128:(qc + 1) * 128], ident[:])
            nc.vector.tensor_add(outbuf[qc][:, i * 128:(i + 1) * 128], ps5[:],
                                 hbuf[qc][:, i * 128:(i + 1) * 128])

    for qc in range(QTT):
        nc.sync.dma_start(p["out"][qc * 128:(qc + 1) * 128, :], outbuf[qc][:])

    s4.close()
    s3.close()


# ======================= host side =======================

_NC_CACHE = None


def _get_module():
    global _NC_CACHE
    if _NC_CACHE is None:
        _NC_CACHE = build_module()
    return _NC_CACHE


def host_prep(inputs):
    """Full inputs -> per-core in_maps (list of 8 dicts)."""
    f32 = np.float32
    x = np.asarray(inputs["x"], f32)
    DA = np.asarray(inputs["DA"])
    g1 = np.asarray(inputs["g1"], f32)
    g2 = np.asarray(inputs["g2"], f32)
    Wq = np.asarray(inputs["Wq"], f32)
    Wk = np.asarray(inputs["Wk"], f32)
    Wv = np.asarray(inputs["Wv"], f32)
    W1 = np.asarray(inputs["W1"], f32)
    W2 = np.asarray(inputs["W2"], f32)
    W3 = np.asarray(inputs["W3"], f32)
    bq = np.asarray(inputs["bq"], f32)
    bk = np.asarray(inputs["bk"], f32)
    bv = np.asarray(inputs["bv"], f32)
    b1 = np.asarray(inputs["b1"], f32)
    b2 = np.asarray(inputs["b2"], f32)
    b3 = np.asarray(inputs["b3"], f32)

    wcast = (lambda a: np.ascontiguousarray(a).astype(ml_dtypes.bfloat16)) \
        if KDT == "bf16" else (lambda a: np.ascontiguousarray(a.astype(np.float32)))
    C = np.ascontiguousarray
    s = 1.0 / np.sqrt(HD)
    shared = {
        "wqT": wcast((Wq * g1[None, :]).T * s),
        "bq8": C((bq * s)[:, None]),
        "wkT": wcast((Wk * g1[None, :]).T),
        "bk": C(bk[:, None]),
        "wvT": wcast((Wv * g1[None, :]).T),
        "w1T": wcast((W1 * g2[None, :]).T),
        "b1": C(b1[:, None]),
        "w2T": wcast((W2 * g2[None, :]).T),
        "b2": C(b2[:, None]),
        "w3T": wcast(W3.T),
        "b3": C(b3[:, None]),
    }
    maskT = [(DA[b, 0] != 0).astype(ml_dtypes.bfloat16).T for b in range(B)]

    in_maps = []
    for c in range(NCORES):
        b = c // (NCORES // B)
        qs = (c % (NCORES // B)) * QT
        xo = x[b, qs:qs + QT]
        in_maps.append(dict(
            shared,
            xf=C(x[b]),
            xo=C(xo),
            xb=C(xo + bv[None, :]),
            mT=C(maskT[b][:, qs:qs + QT]),
        ))
    return in_maps


def assemble(results):
    out = np.empty((B, N, D), np.float32)
    for c in range(NCORES):
        b = c // (NCORES // B)
        qs = (c % (NCORES // B)) * QT
        out[b, qs:qs + QT] = results[c]["out"]
    return out


LAST_EXEC_NS = None


def kernel(_trace=False, **inputs):
    from concourse.bass_utils import run_bass_kernel_spmd

    global LAST_EXEC_NS
    nc = _get_module()
    in_maps = host_prep(inputs)
    res = run_bass_kernel_spmd(nc, in_maps, list(range(NCORES)), trace=_trace)
    LAST_EXEC_NS = res.exec_time_ns
    return assemble(res.results)



# revision 7
# speedup vs baseline: 1.2455x; 1.2455x over previous
"""Trainium2 Bass kernel for the GAT block (masked attention + SwiGLU MLP).

Sharding: token-split across 8 cores. Core c handles batch b = c//4 and the
512-query slice starting at (c%4)*512 of that batch. Each core computes
full-batch K/V projections (duplicated across the 4 cores of a batch -- no
collectives), its own queries' attention, and the MLP for its token slice.

Device-side strategy:
  - activations token-major [tokens, d] for normalizations (free-dim
    reductions, per-partition scales), PE-transposed to feature-major
    [d, tokens] where they feed matmul contractions.
  - attention scores computed TRANSPOSED: sT[keys, queries], so exp/mask
    need no reductions; p = exp(sT)*mask feeds AV as the STATIONARY operand
    per [128k x 128q] tile, with v65 moving -> AV outputs land [queries, 65]
    with full 128-row contraction (half the PE cycles of the moving-p form)
    and the softmax denominator (ones column of v65) arrives per-partition.
  - no max-subtraction in softmax: scores are O(6) for this input
    distribution, exp is safe, softmax is shift-invariant.
  - host folds g1/g2 into weights, 1/sqrt(hd) into Wq/bq, bv into the
    attention residual; remaining biases fold into PSUM-evacuation ops.
  - DMA order: x tiles first (they gate the front), then K/V/Q weights,
    masks, then MLP weights during attention.
  - engine balance: exp (the Act floor) owns Activation; evacuations and
    elementwise go to DVE/Pool.
"""

import os
import sys

sys.path.insert(0, "/opt/trn_rl_repo")

# CoreSim doesn't implement Silu; sim runs decompose it into Sigmoid+mul.
SIM_SILU = os.environ.get("KSIM_SILU") == "1"
# weight/activation compute dtype for projections+MLP: bf16 (default) or f32r
KDT = os.environ.get("KDT", "bf16")

from contextlib import ExitStack

import ml_dtypes
import numpy as np

import concourse.bass as bass
import concourse.mybir as mybir
import concourse.tile as tile
from concourse import bacc
from concourse.masks import make_identity

D = 512
N = 2048
B = 2
HEADS = 8
HD = 64
HDIM = 2048
NCORES = 8
QT = 512  # tokens (queries) per core
EPS = float(np.finfo(np.float32).eps)

F32 = mybir.dt.float32
F32R = mybir.dt.float32r
BF16 = mybir.dt.bfloat16

AF = mybir.ActivationFunctionType
ALU = mybir.AluOpType

WDT = BF16 if KDT == "bf16" else F32R
ZTDT = BF16  # pre-transpose z tiles / transpose PSUM

DT4 = D // 128    # 4 feature tiles
TT = N // 128     # 16 token tiles (full batch)
QTT = QT // 128   # 4 own-query tiles
HT = HDIM // 128  # 16 hidden tiles
KC = N // 512     # 4 key chunks of 512


def build_module(reps=1):
    nc = bacc.Bacc(
        "TRN2", target_bir_lowering=False, debug=False, num_devices=NCORES)

    p = {}
    def param(name, shape, dtype=F32, out=False):
        p[name] = nc.declare_dram_parameter(name, shape, dtype, isOutput=out)
        return p[name]

    param("xf", [N, D])            # full batch x
    param("xo", [QT, D])           # own-slice x
    param("xb", [QT, D])           # own-slice x + bv (residual base)
    param("mT", [N, QT], BF16)     # mask transposed [keys, queries], 0/1
    param("wqT", [D, D], WDT)           # (Wq*g1).T / 8
    param("bq8", [D, 1])           # bq / 8
    param("wkT", [D, D], WDT)           # (Wk*g1).T
    param("bk", [D, 1])
    param("wvT", [D, D], WDT)           # (Wv*g1).T
    param("w1T", [D, HDIM], WDT)        # (W1*g2).T
    param("b1", [HDIM, 1])
    param("w2T", [D, HDIM], WDT)        # (W2*g2).T
    param("b2", [HDIM, 1])
    param("w3T", [HDIM, D], WDT)        # W3.T
    param("b3", [D, 1])
    param("out", [QT, D], out=True)

    with ExitStack() as ctx:
        tc = ctx.enter_context(tile.TileContext(nc))
        for _ in range(reps):
            with ExitStack() as rctx:
                _body(rctx, tc, nc, p)
    nc.compile()
    return nc


def _body(ctx, tc, nc, p):
    # ---------- long-lived pools ----------
    persist = ctx.enter_context(tc.tile_pool(name="persist", bufs=1))
    small = ctx.enter_context(tc.tile_pool(name="small", bufs=8))

    ident = persist.tile([128, 128], F32, tag="ident", name="ident")
    make_identity(nc, ident[:])
    identw = persist.tile([128, 128], ZTDT, tag="identw", name="identw")
    nc.vector.tensor_copy(identw[:], ident[:])
    epsb = persist.tile([128, 1], F32, tag="epsb", name="epsb")
    nc.gpsimd.memset(epsb[:], EPS)

    xb_s = [persist.tile([128, D], F32, tag=f"xb{q}", name=f"xb{q}") for q in range(QTT)]
    hbuf = [persist.tile([128, D], F32, tag=f"hb{q}", name=f"hb{q}") for q in range(QTT)]

    # ================= scope 1: front (z, zT) + projections ========
    s1 = ExitStack()
    wpool = s1.enter_context(tc.tile_pool(name="wqkv", bufs=1))
    front = s1.enter_context(tc.tile_pool(name="front", bufs=6))
    mm_ps = s1.enter_context(tc.tile_pool(name="mm_ps", bufs=3, space="PSUM"))

    # x tiles stream in FIRST (they gate the whole front); weights follow.
    xts = []
    for t in range(TT + QTT):
        xt = front.tile([128, D], F32, tag="xt", name="xt")
        src = p["xf"] if t < TT else p["xo"]
        row0 = t * 128 if t < TT else (t - TT) * 128
        nc.sync.dma_start(xt[:], src[row0:row0 + 128, :])
        xts.append(xt)

    wq_s = [wpool.tile([128, D], WDT, tag=f"wq{i}", name=f"wq{i}") for i in range(DT4)]
    wk_s = [wpool.tile([128, D], WDT, tag=f"wk{i}", name=f"wk{i}") for i in range(DT4)]
    wv_s = [wpool.tile([128, D], WDT, tag=f"wv{i}", name=f"wv{i}") for i in range(DT4)]
    for i in range(DT4):
        nc.sync.dma_start(wk_s[i][:], p["wkT"][i * 128:(i + 1) * 128, :])
        nc.sync.dma_start(wv_s[i][:], p["wvT"][i * 128:(i + 1) * 128, :])
        nc.sync.dma_start(wq_s[i][:], p["wqT"][i * 128:(i + 1) * 128, :])
    bq_s = [small.tile([128, 1], F32, tag=f"bqs{i}", name=f"bqs{i}") for i in range(DT4)]
    bk_s = [small.tile([128, 1], F32, tag=f"bks{i}", name=f"bks{i}") for i in range(DT4)]
    for i in range(DT4):
        nc.sync.dma_start(bq_s[i][:], p["bq8"][i * 128:(i + 1) * 128, :])
        nc.sync.dma_start(bk_s[i][:], p["bk"][i * 128:(i + 1) * 128, :])

    # single tensors, d-major chunks: zT_all[:, d*N + col], zoT_all[:, d*QT + col]
    zT_all = wpool.tile([128, DT4 * N], WDT, tag="zT_all", name="zT_all")
    zoT_all = wpool.tile([128, DT4 * QT], WDT, tag="zoT_all", name="zoT_all")
    zT = [zT_all[:, d * N:(d + 1) * N] for d in range(DT4)]
    zoT = [zoT_all[:, d * QT:(d + 1) * QT] for d in range(DT4)]

    def norm_group(g, tiles, zT_dst_all, ncols):
        """rmsnorm + transpose a group of 4 token tiles into zT_dst_all."""
        G = len(tiles)
        sss = small.tile([128, G], F32, tag="sss", name="sss")
        srtg = small.tile([128, G], F32, tag="srtg", name="srtg")
        invg = small.tile([128, G], F32, tag="invg", name="invg")
        for i, (t, xt) in enumerate(tiles):
            scr = front.tile([128, D], BF16, tag="rms_scr", name="rms_scr")
            nc.vector.scalar_tensor_tensor(
                out=scr[:], in0=xt[:], scalar=1.0, in1=xt[:],
                op0=ALU.mult, op1=ALU.mult, accum_out=sss[:, i:i + 1])
        nc.scalar.activation(srtg[:], sss[:], AF.Sqrt, bias=epsb[:], scale=1.0 / D)
        nc.vector.reciprocal(invg[:], srtg[:])
        for i, (t, xt) in enumerate(tiles):
            zt = front.tile([128, D], ZTDT, tag="zt", name="zt")
            if t % 2 == 0:
                nc.scalar.activation(zt[:], xt[:], AF.Copy, scale=invg[:, i:i + 1])
            else:
                nc.gpsimd.tensor_scalar_mul(zt[:], xt[:], invg[:, i:i + 1])
            ps = mm_ps.tile([128, 512], ZTDT, tag="mm", name="mm", bufs=2)
            for d in range(DT4):
                nc.tensor.matmul(ps[:, d * 128:(d + 1) * 128],
                                 zt[:, d * 128:(d + 1) * 128], identw[:],
                                 is_transpose=True,
                                 start=(d == 0), stop=(d == DT4 - 1))
            dst = zT_dst_all[:].rearrange(
                "p (d c) -> p d c", c=ncols)[:, :, t * 128:(t + 1) * 128]
            eng = nc.scalar.copy if t % 2 == 0 else nc.vector.tensor_copy
            eng(dst, ps[:].rearrange("p (d c) -> p d c", c=128))

    # ---------- attention operand pools (filled during the front) ----------
    s2 = ExitStack()
    apool = s2.enter_context(tc.tile_pool(name="attn", bufs=1, side="right"))
    arot = s2.enter_context(tc.tile_pool(name="arot", bufs=4, side="right"))

    kT = [apool.tile([128, N], BF16, tag=f"kT{pr}", name=f"kT{pr}") for pr in range(DT4)]
    qT = [apool.tile([128, QT], BF16, tag=f"qT{pr}", name=f"qT{pr}") for pr in range(DT4)]
    v65_all = apool.tile([128, TT * HEADS * (HD + 1)], BF16, tag="v65_all", name="v65_all")
    v65 = [v65_all[:, t * HEADS * (HD + 1):(t + 1) * HEADS * (HD + 1)] for t in range(TT)]
    nc.vector.memset(
        v65_all[:].rearrange("q (t h c) -> q t h c", t=TT, c=HD + 1)[:, :, :, HD:HD + 1],
        1.0)

    # interleave: normalize 4 xf tiles -> kT chunk g + v65 group g
    for g in range(TT // 4):
        norm_group(g, [(t, xts[t]) for t in range(g * 4, g * 4 + 4)], zT_all, N)
        # kT projection for key chunk g, all 4 partition-rows
        for pr in range(DT4):
            ps = mm_ps.tile([128, 512], F32, tag="pk", name="pk", bufs=2)
            for dk in range(DT4):
                nc.tensor.matmul(ps[:],
                                 wk_s[dk][:, pr * 128:(pr + 1) * 128],
                                 zT[dk][:, g * 512:(g + 1) * 512],
                                 start=(dk == 0), stop=(dk == DT4 - 1))
            if pr % 2 == 0:
                nc.scalar.activation(kT[pr][:, g * 512:(g + 1) * 512], ps[:],
                                     AF.Identity, bias=bk_s[pr][:], scale=1.0)
            else:
                nc.vector.tensor_scalar_add(kT[pr][:, g * 512:(g + 1) * 512],
                                            ps[:], bk_s[pr][:])
        # v projection for token tiles of group g
        ps = mm_ps.tile([128, 2048], F32, tag="pv", name="pv", bufs=1)
        for tt in range(4):
            t = g * 4 + tt
            for dk in range(DT4):
                nc.tensor.matmul(ps[:, tt * 512:(tt + 1) * 512],
                                 zT[dk][:, t * 128:(t + 1) * 128], wv_s[dk][:],
                                 start=(dk == 0), stop=(dk == DT4 - 1))
        dst = v65_all[:, g * 4 * HEADS * (HD + 1):(g + 1) * 4 * HEADS * (HD + 1)]
        eng = nc.vector.tensor_copy if g % 2 == 0 else nc.scalar.copy
        eng(dst.rearrange("q (t h c) -> q t h c", t=4, c=HD + 1)[:, :, :, 0:HD],
            ps[:].rearrange("q (t h c) -> q t h c", t=4, c=HD))

    # own-slice queries
    for g in range(QTT // 4):
        norm_group(g, [(t, xts[TT + t]) for t in range(g * 4, g * 4 + 4)],
                   zoT_all, QT)
    for pr in range(DT4):
        ps = mm_ps.tile([128, 512], F32, tag="pk", name="pk", bufs=2)
        for dk in range(DT4):
            nc.tensor.matmul(ps[:], wq_s[dk][:, pr * 128:(pr + 1) * 128],
                             zoT[dk][:], start=(dk == 0), stop=(dk == DT4 - 1))
        if pr % 2 == 0:
            nc.scalar.activation(qT[pr][:], ps[:], AF.Identity,
                                 bias=bq_s[pr][:], scale=1.0)
        else:
            nc.vector.tensor_scalar_add(qT[pr][:], ps[:], bq_s[pr][:])

    # masks + residual base: needed from attention start; MLP weights later
    mt2 = [apool.tile([128, 2 * QT], BF16, tag=f"mt2_{g}", name=f"mt2_{g}")
           for g in range(TT // 2)]
    for g in range(TT // 2):
        nc.sync.dma_start(
            mt2[g][:].rearrange("p (a q) -> p a q", a=2),
            p["mT"][g * 256:(g + 1) * 256, :].rearrange("(a p) q -> p a q", p=128))
    for q in range(QTT):
        nc.sync.dma_start(xb_s[q][:], p["xb"][q * 128:(q + 1) * 128, :])

    s1.close()  # frees wqkv/front zones (zT, zoT, wq/wk/wv) + mm_ps banks

    s2b = ExitStack()
    sc_ps = s2b.enter_context(tc.tile_pool(name="sc_ps", bufs=3, space="PSUM", side="right"))
    av_ps = s2b.enter_context(tc.tile_pool(name="av_ps", bufs=2, space="PSUM", side="right"))

    # ---- MLP weights: load during attention into the freed zone ----
    s3 = ExitStack()
    w12pool = s3.enter_context(tc.tile_pool(name="w12", bufs=1))
    w1_s = [w12pool.tile([128, HDIM], WDT, tag=f"w1{i}", name=f"w1{i}") for i in range(DT4)]
    w2_s = [w12pool.tile([128, HDIM], WDT, tag=f"w2{i}", name=f"w2{i}") for i in range(DT4)]
    w3_s = [w12pool.tile([128, D], WDT, tag=f"w3{j}", name=f"w3{j}") for j in range(HT)]
    for i in range(DT4):
        nc.sync.dma_start(w1_s[i][:], p["w1T"][i * 128:(i + 1) * 128, :])
        nc.sync.dma_start(w2_s[i][:], p["w2T"][i * 128:(i + 1) * 128, :])
    for j in range(HT):
        nc.sync.dma_start(w3_s[j][:], p["w3T"][j * 128:(j + 1) * 128, :])
    b1_s = [small.tile([128, 1], F32, tag=f"b1t{j}", name=f"b1t{j}") for j in range(HT)]
    b2_s = [small.tile([128, 1], F32, tag=f"b2t{j}", name=f"b2t{j}") for j in range(HT)]
    for j in range(HT):
        nc.sync.dma_start(b1_s[j][:], p["b1"][j * 128:(j + 1) * 128, :])
        nc.sync.dma_start(b2_s[j][:], p["b2"][j * 128:(j + 1) * 128, :])

    # ================= attention =================
    # per head-pair pr: scores sT[keys, 512q] via 64-row matmuls packed in the
    # PE array, exp+mask -> p_t; AV with p-tile stationary: out[128q, 65].
    for pr in range(DT4):
        p_t = [apool.tile([128, TT * 512], BF16, tag=f"p{sub}", name=f"p{sub}") for sub in (0, 1)]
        for g in range(TT // 2):  # kt groups of 2
            ps_pair = []
            for sub in (0, 1):
                ps_s = sc_ps.tile([128, 1024], F32, tag="sc", name="sc")
                ps_pair.append(ps_s)
            for half in (0, 1):
                kt = 2 * g + half
                for sub in (0, 1):
                    lhsT = kT[pr][64 * sub:64 * (sub + 1), kt * 128:(kt + 1) * 128]
                    rhs = qT[pr][64 * sub:64 * (sub + 1), :]
                    nc.tensor.matmul(ps_pair[sub][:, half * 512:(half + 1) * 512],
                                     lhsT, rhs, start=True, stop=True,
                                     tile_position=(64 * sub, 0))
            for sub in (0, 1):
                praw = arot.tile([128, 1024], BF16, tag="praw", name="praw")
                nc.scalar.activation(praw[:], ps_pair[sub][:], AF.Exp,
                                     bias=0.0, scale=1.0)
                nc.vector.tensor_mul(p_t[sub][:, g * 1024:(g + 1) * 1024],
                                     praw[:], mt2[g][:])
        for sub in (0, 1):
            h = 2 * pr + sub
            av = av_ps.tile([128, QTT * (HD + 1)], F32, tag="av", name="av")
            for qc in range(QTT):
                for kt in range(TT):
                    nc.tensor.matmul(
                        av[:, qc * (HD + 1):(qc + 1) * (HD + 1)],
                        p_t[sub][:, kt * 512 + qc * 128:kt * 512 + (qc + 1) * 128],
                        v65[kt][:, (HD + 1) * h:(HD + 1) * (h + 1)],
                        start=(kt == 0), stop=(kt == TT - 1))
            for qc in range(QTT):
                rec = small.tile([128, 1], F32, tag="rec", name="rec")
                nc.vector.reciprocal(
                    rec[:], av[:, qc * (HD + 1) + HD:qc * (HD + 1) + HD + 1])
                nc.vector.scalar_tensor_tensor(
                    out=hbuf[qc][:, HD * h:HD * (h + 1)],
                    in0=av[:, qc * (HD + 1):qc * (HD + 1) + HD],
                    scalar=rec[:], in1=xb_s[qc][:, HD * h:HD * (h + 1)],
                    op0=ALU.mult, op1=ALU.add)

    s2b.close()
    s2.close()  # frees kT/qT/v65/mask/p zones

    # ================= hn + MLP =================
    s4 = ExitStack()
    mpool = s4.enter_context(tc.tile_pool(name="mlp", bufs=1))
    mrot = s4.enter_context(tc.tile_pool(name="mrot", bufs=3))
    mm_ps = s4.enter_context(tc.tile_pool(name="mm_ps2", bufs=3, space="PSUM"))
    w3_ps = s4.enter_context(tc.tile_pool(name="w3_ps", bufs=1, space="PSUM"))
    hnT_all = mpool.tile([128, DT4 * QT], WDT, tag="hnT_all", name="hnT_all")
    hnT = [hnT_all[:, d * QT:(d + 1) * QT] for d in range(DT4)]
    gbuf = [mpool.tile([128, QT], WDT, tag=f"g{j}", name=f"g{j}") for j in range(HT)]
    outbuf = [mpool.tile([128, D], F32, tag=f"ob{q}", name=f"ob{q}") for q in range(QTT)]

    for qc in range(QTT):
        scr = mrot.tile([128, D], BF16, tag="rms_scr", name="rms_scr")
        ssq = small.tile([128, 1], F32, tag="ssq", name="ssq")
        nc.vector.scalar_tensor_tensor(
            out=scr[:], in0=hbuf[qc][:], scalar=1.0, in1=hbuf[qc][:],
            op0=ALU.mult, op1=ALU.mult, accum_out=ssq[:])
        srt = small.tile([128, 1], F32, tag="srt", name="srt")
        nc.scalar.activation(srt[:], ssq[:], AF.Sqrt, bias=epsb[:], scale=1.0 / D)
        inv2 = small.tile([128, 1], F32, tag="inv", name="inv")
        nc.vector.reciprocal(inv2[:], srt[:])
        z2 = mrot.tile([128, D], ZTDT, tag="z2", name="z2")
        nc.gpsimd.tensor_scalar_mul(z2[:], hbuf[qc][:], inv2[:])
        ps = mm_ps.tile([128, 512], ZTDT, tag="mm", name="mm")
        for d in range(DT4):
            nc.tensor.matmul(ps[:, d * 128:(d + 1) * 128],
                             z2[:, d * 128:(d + 1) * 128], identw[:],
                             is_transpose=True, start=(d == 0), stop=(d == DT4 - 1))
        eng = nc.scalar.copy if qc % 2 == 0 else nc.vector.tensor_copy
        eng(hnT_all[:].rearrange("p (d c) -> p d c", c=QT)[:, :, qc * 128:(qc + 1) * 128],
            ps[:].rearrange("p (d c) -> p d c", c=128))

    # j loop with W3 accumulation software-pipelined 2 deep
    ps4 = w3_ps.tile([128, 2048], F32, tag="w3acc", name="w3acc")

    def w3_step(j):
        for i in range(DT4):
            nc.tensor.matmul(ps4[:, i * 512:(i + 1) * 512],
                             w3_s[j][:, i * 128:(i + 1) * 128], gbuf[j][:],
                             start=(j == 0), stop=(j == HT - 1))

    for j in range(HT):
        ps2 = mm_ps.tile([128, 512], F32, tag="mm", name="mm")
        for dk in range(DT4):
            nc.tensor.matmul(ps2[:], w1_s[dk][:, j * 128:(j + 1) * 128], hnT[dk][:],
                             start=(dk == 0), stop=(dk == DT4 - 1))
        su = mrot.tile([128, 512], F32, tag="su", name="su")
        if SIM_SILU:
            a2 = mrot.tile([128, 512], F32, tag="a2", name="a2")
            nc.scalar.activation(a2[:], ps2[:], AF.Identity, bias=b1_s[j][:], scale=1.0)
            sg = mrot.tile([128, 512], F32, tag="sg", name="sg")
            nc.scalar.activation(sg[:], ps2[:], AF.Sigmoid, bias=b1_s[j][:], scale=1.0)
            nc.vector.tensor_mul(su[:], a2[:], sg[:])
        else:
            nc.scalar.activation(su[:], ps2[:], AF.Silu, bias=b1_s[j][:], scale=1.0)
        ps3 = mm_ps.tile([128, 512], F32, tag="mm", name="mm")
        for dk in range(DT4):
            nc.tensor.matmul(ps3[:], w2_s[dk][:, j * 128:(j + 1) * 128], hnT[dk][:],
                             start=(dk == 0), stop=(dk == DT4 - 1))
        nc.vector.scalar_tensor_tensor(
            out=gbuf[j][:], in0=ps3[:], scalar=b2_s[j][:], in1=su[:],
            op0=ALU.add, op1=ALU.mult)
        if j >= 2:
            w3_step(j - 2)
    w3_step(HT - 2)
    w3_step(HT - 1)

    b3_s = [small.tile([128, 1], F32, tag=f"b3t{i}", name=f"b3t{i}") for i in range(DT4)]
    for i in range(DT4):
        nc.sync.dma_start(b3_s[i][:], p["b3"][i * 128:(i + 1) * 128, :])
    for i in range(DT4):
        outT = mrot.tile([128, 512], ZTDT, tag="outT", name="outT")
        nc.scalar.activation(outT[:], ps4[:, i * 512:(i + 1) * 512],
                             AF.Identity, bias=b3_s[i][:], scale=1.0)
        for qc in range(QTT):
            ps5 = mm_ps.tile([128, 128], ZTDT, tag="mm", name="mm")
            nc.tensor.matmul(ps5[:], outT[:, qc * 128:(qc + 1) * 128], identw[:],
                             is_transpose=True, start=True, stop=True)
            nc.vector.tensor_add(outbuf[qc][:, i * 128:(i + 1) * 128], ps5[:],
                                 hbuf[qc][:, i * 128:(i + 1) * 128])
    for qc in range(QTT):
        nc.sync.dma_start(p["out"][qc * 128:(qc + 1) * 128, :], outbuf[qc][:])

    s4.close()
    s3.close()


# ======================= host side =======================

_NC_CACHE = None


def _get_module():
    global _NC_CACHE
    if _NC_CACHE is None:
        _NC_CACHE = build_module()
    return _NC_CACHE


def host_prep(inputs):
    """Full inputs -> per-core in_maps (list of 8 dicts)."""
    f32 = np.float32
    x = np.asarray(inputs["x"], f32)
    DA = np.asarray(inputs["DA"])
    g1 = np.asarray(inputs["g1"], f32)
    g2 = np.asarray(inputs["g2"], f32)
    Wq = np.asarray(inputs["Wq"], f32)
    Wk = np.asarray(inputs["Wk"], f32)
    Wv = np.asarray(inputs["Wv"], f32)
    W1 = np.asarray(inputs["W1"], f32)
    W2 = np.asarray(inputs["W2"], f32)
    W3 = np.asarray(inputs["W3"], f32)
    bq = np.asarray(inputs["bq"], f32)
    bk = np.asarray(inputs["bk"], f32)
    bv = np.asarray(inputs["bv"], f32)
    b1 = np.asarray(inputs["b1"], f32)
    b2 = np.asarray(inputs["b2"], f32)
    b3 = np.asarray(inputs["b3"], f32)

    wcast = (lambda a: np.ascontiguousarray(a).astype(ml_dtypes.bfloat16)) \
        if KDT == "bf16" else (lambda a: np.ascontiguousarray(a.astype(np.float32)))
    C = np.ascontiguousarray
    s = 1.0 / np.sqrt(HD)
    shared = {
        "wqT": wcast((Wq * g1[None, :]).T * s),
        "bq8": C((bq * s)[:, None]),
        "wkT": wcast((Wk * g1[None, :]).T),
        "bk": C(bk[:, None]),
        "wvT": wcast((Wv * g1[None, :]).T),
        "w1T": wcast((W1 * g2[None, :]).T),
        "b1": C(b1[:, None]),
        "w2T": wcast((W2 * g2[None, :]).T),
        "b2": C(b2[:, None]),
        "w3T": wcast(W3.T),
        "b3": C(b3[:, None]),
    }
    maskT = [(DA[b, 0] != 0).astype(ml_dtypes.bfloat16).T for b in range(B)]

    in_maps = []
    for c in range(NCORES):
        b = c // (NCORES // B)
        qs = (c % (NCORES // B)) * QT
        xo = x[b, qs:qs + QT]
        in_maps.append(dict(
            shared,
            xf=C(x[b]),
            xo=C(xo),
            xb=C(xo + bv[None, :]),
            mT=C(maskT[b][:, qs:qs + QT]),
        ))
    return in_maps


def assemble(results):
    out = np.empty((B, N, D), np.float32)
    for c in range(NCORES):
        b = c // (NCORES // B)
        qs = (c % (NCORES // B)) * QT
        out[b, qs:qs + QT] = results[c]["out"]
    return out


LAST_EXEC_NS = None


def kernel(_trace=False, **inputs):
    from concourse.bass_utils import run_bass_kernel_spmd

    global LAST_EXEC_NS
    nc = _get_module()
    in_maps = host_prep(inputs)
    res = run_bass_kernel_spmd(nc, in_maps, list(range(NCORES)), trace=_trace)
    LAST_EXEC_NS = res.exec_time_ns
    return assemble(res.results)
